# revision 1
# baseline (speedup 1.0000x reference)
"""Trainium2 Bass kernel for nn_Discriminator2 (bilinear discriminator scores).

Math: with hc0 = h_c[0] [N, D], W0 = W[0] [D, D]:
    v      = hc0 @ W0.T                      [N, D]   (tensor engine)
    sc1[n] = dot(h_pl[0][n], v[n]) + b       [N]      (fused DVE mult+reduce)
    sc2[s,n] = dot(hc0[sample[s,n]], v[n]) + b        (indirect-DMA gather + DVE)
    out    = [sc1 | sc2.flat | sc2.flat]     [1, N + 2*S*N]

Sharding: nodes (N) split evenly across 8 cores; hc0 replicated on every core
so gathers resolve locally; W replicated; h_pl / sample_list sharded by node.
"""

import sys

for _p in ("/opt/trn_rl_repo",):
    if _p not in sys.path:
        sys.path.insert(0, _p)

import numpy as np

import concourse.bass as bass
import concourse.mybir as mybir
import concourse.tile as tile
from concourse import bacc
from concourse.bass_utils import run_bass_kernel_spmd

P = 128  # partitions


class Cfg:
    """Problem geometry. Full-size defaults; shrink for CoreSim validation."""

    def __init__(self, n_table=100000, nodes_per_core=12500, d=512, s=4,
                 n_cores=8, super_tile=4, mm_dtype=mybir.dt.float32r):
        self.n_table = n_table          # rows of the gather table (full N)
        self.nodes_per_core = nodes_per_core
        self.d = d
        self.s = s
        self.n_cores = n_cores
        self.super_tile = super_tile    # node-tiles per hcT DMA block
        self.mm_dtype = mm_dtype
        self.tiles = -(-nodes_per_core // P)        # ceil
        self.npad = self.tiles * P
        self.kc = d // P                # contraction chunks


FULL = Cfg()


def build_nc(cfg: Cfg):
    D, S, KC, TILES = cfg.d, cfg.s, cfg.kc, cfg.tiles
    mmdt = cfg.mm_dtype
    f32 = mybir.dt.float32

    nc = bacc.Bacc("TRN2", target_bir_lowering=False, debug=False,
                   num_swdge_queues=2)
    hc = nc.dram_tensor("hc", [cfg.n_table, D], f32, kind="ExternalInput").ap()
    hcT = nc.dram_tensor("hcT", [D, cfg.npad], mmdt, kind="ExternalInput").ap()
    hpl = nc.dram_tensor("hpl", [cfg.npad, D], f32, kind="ExternalInput").ap()
    idx = nc.dram_tensor("idx", [P, TILES * S], mybir.dt.int32,
                         kind="ExternalInput").ap()
    wt = nc.dram_tensor("wt", [D, D], mmdt, kind="ExternalInput").ap()
    bb = nc.dram_tensor("bb", [P, 1], f32, kind="ExternalInput").ap()
    out = nc.dram_tensor("out", [P, TILES * (S + 1)], f32,
                         kind="ExternalOutput").ap()

    with tile.TileContext(nc) as tc:
        with (
            tc.tile_pool(name="const", bufs=1) as cpool,
            tc.tile_pool(name="hcT", bufs=2) as hcT_pool,
            tc.tile_pool(name="hpl", bufs=4) as hpl_pool,
            tc.tile_pool(name="g", bufs=6) as g_pool,
            tc.tile_pool(name="prod", bufs=8) as prod_pool,
            tc.tile_pool(name="psum", bufs=4, space="PSUM") as psum_pool,
        ):
            # All gather indices resident: idx_sb[p, t*S+s] = sample[s, t*128+p].
            # Loaded FIRST so the gather stream (the kernel's critical path)
            # starts as early as possible.
            idx_sb = cpool.tile([P, TILES * S], mybir.dt.int32)
            nc.sync.dma_start(out=idx_sb[:], in_=idx[:])
            # W.T resident: free layout (c, d) — chunk c covers contraction
            # rows c*128..c*128+127.
            wt_sb = cpool.tile([P, KC * D], mmdt)
            nc.sync.dma_start(
                out=wt_sb[:].rearrange("p (c d) -> p c d", c=KC),
                in_=wt.rearrange("(c p) d -> p c d", p=P))
            b_sb = cpool.tile([P, 1], f32)
            nc.sync.dma_start(out=b_sb[:], in_=bb[:])
            sc_acc = cpool.tile([P, TILES * (S + 1)], f32)
            dump = cpool.tile([P, D], f32)  # discarded ACT elementwise output

            for t0 in range(0, TILES, cfg.super_tile):
                st = min(cfg.super_tile, TILES - t0)
                # hcT block [D, st*128] -> SBUF free layout (c, n_local)
                hcT_sb = hcT_pool.tile([P, KC * cfg.super_tile * P], mmdt,
                                       tag="hcT")
                nc.sync.dma_start(
                    out=hcT_sb[:, : KC * st * P].rearrange(
                        "p (c n) -> p c n", c=KC),
                    in_=hcT[:, t0 * P:(t0 + st) * P].rearrange(
                        "(c p) n -> p c n", p=P),
                )
                for j in range(st):
                    t = t0 + j
                    hpl_sb = hpl_pool.tile([P, D], f32, tag="hpl")
                    nc.sync.dma_start(out=hpl_sb[:],
                                      in_=hpl[t * P:(t + 1) * P, :])
                    # Gather the S sampled rows per node (HW indirect DMA
                    # honors one index per partition, so one call per s):
                    # g_sb[p, s*D:(s+1)*D] = hc[idx_sb[p, t*S+s], :]
                    g_sb = g_pool.tile([P, S * D], f32, tag="g")
                    for s in range(S):
                        gi = nc.gpsimd.indirect_dma_start(
                            out=g_sb[:, s * D:(s + 1) * D],
                            out_offset=None,
                            in_=hc[:],
                            in_offset=bass.IndirectOffsetOnAxis(
                                ap=idx_sb[:, t * S + s:t * S + s + 1], axis=0),
                        )
                        # alternate SWDGE queues so SDMA interleaves two
                        # descriptor streams (hides random-row HBM latency)
                        if s % 2 == 1:
                            gi.ins.queue = "qPoolDynamic1"
                    # v = hc0_tile @ W.T via 4 accumulating matmuls
                    v_ps = psum_pool.tile([P, D], f32, space="PSUM", tag="v_ps")
                    for c in range(KC):
                        off = (c * st + j) * P
                        nc.tensor.matmul(
                            out=v_ps[:],
                            lhsT=hcT_sb[:, off:off + P],
                            rhs=wt_sb[:, c * D:(c + 1) * D],
                            start=(c == 0),
                            stop=(c == KC - 1),
                        )
                    # 5 dot products: DVE multiplies (v read straight from
                    # PSUM), ScalarE reduces via Copy-activation accum_out.
                    for s in range(S + 1):
                        in0 = hpl_sb[:] if s == 0 else g_sb[:, (s - 1) * D:s * D]
                        prod = prod_pool.tile([P, D], f32, tag="prod")
                        nc.vector.tensor_mul(prod[:], in0, v_ps[:])
                        nc.scalar.activation(
                            dump[:], prod[:],
                            mybir.ActivationFunctionType.Copy,
                            accum_out=sc_acc[:, t * (S + 1) + s:
                                             t * (S + 1) + s + 1],
                        )
            nc.vector.tensor_scalar_add(sc_acc[:], sc_acc[:], b_sb[:, :1])
            nc.sync.dma_start(out=out[:], in_=sc_acc[:])
    nc.compile()
    return nc


def make_in_maps(cfg: Cfg, h_c, h_pl, sample_list, W, b):
    """Host-side sharding: full inputs -> per-core input dicts."""
    D, S = cfg.d, cfg.s
    hc0 = np.ascontiguousarray(np.asarray(h_c, np.float32)[0])
    hpl0 = np.asarray(h_pl, np.float32)[0]
    smp = np.asarray(sample_list)
    W0 = np.asarray(W, np.float32)[0]
    bval = float(np.asarray(b, np.float32).reshape(-1)[0])

    hcT = np.ascontiguousarray(hc0.T)                  # [D, N]
    wt = np.ascontiguousarray(W0.T)                    # wt[e, d] = W[d, e]
    b_bcast = np.full((P, 1), bval, np.float32)

    in_maps = []
    for c in range(cfg.n_cores):
        lo = c * cfg.nodes_per_core
        hi = lo + cfg.nodes_per_core
        hcT_s = np.zeros((D, cfg.npad), np.float32)
        hcT_s[:, : cfg.nodes_per_core] = hcT[:, lo:hi]
        hpl_s = np.zeros((cfg.npad, D), np.float32)
        hpl_s[: cfg.nodes_per_core] = hpl0[lo:hi]
        idx_s = np.zeros((S, cfg.npad), np.int64)
        idx_s[:, : cfg.nodes_per_core] = smp[:, lo:hi]
        idx_r = np.ascontiguousarray(
            idx_s.reshape(S, cfg.tiles, P).transpose(2, 1, 0)
            .astype(np.int32).reshape(P, cfg.tiles * S))
        in_maps.append({
            "hc": hc0, "hcT": hcT_s, "hpl": hpl_s,
            "idx": idx_r, "wt": wt, "bb": b_bcast,
        })
    return in_maps


def assemble_output(cfg: Cfg, outs):
    """Per-core 'out' arrays [P, TILES*(S+1)] -> full logits [1, N + 2*S*N]."""
    S = cfg.s
    n = cfg.nodes_per_core * cfg.n_cores
    sc1 = np.empty((n,), np.float32)
    sc2 = np.empty((S, n), np.float32)
    for c in range(cfg.n_cores):
        o = (outs[c].reshape(P, cfg.tiles, S + 1).transpose(2, 1, 0)
             .reshape(S + 1, cfg.npad)[:, : cfg.nodes_per_core])
        lo = c * cfg.nodes_per_core
        sc1[lo:lo + cfg.nodes_per_core] = o[0]
        sc2[:, lo:lo + cfg.nodes_per_core] = o[1:]
    flat = sc2.reshape(-1)
    return np.concatenate([sc1, flat, flat])[None, :].astype(np.float32)


_NC_CACHE = {}


def _get_nc(cfg: Cfg):
    key = (cfg.n_table, cfg.nodes_per_core, cfg.d, cfg.s, cfg.super_tile,
           cfg.mm_dtype)
    if key not in _NC_CACHE:
        _NC_CACHE[key] = build_nc(cfg)
    return _NC_CACHE[key]


def run_on_hw(cfg: Cfg, inputs, trace=False, trace_kwargs={}):
    nc = _get_nc(cfg)
    in_maps = make_in_maps(cfg, **inputs)
    res = run_bass_kernel_spmd(nc, in_maps, core_ids=list(range(cfg.n_cores)),
                               trace=trace, trace_kwargs=trace_kwargs)
    out = assemble_output(cfg, [r["out"] for r in res.results])
    return out, res


def kernel(h_c, h_pl, sample_list, W, b):
    inputs = dict(h_c=h_c, h_pl=h_pl, sample_list=sample_list, W=W, b=b)
    out, _ = run_on_hw(FULL, inputs, trace=False)
    return out

